# revision 24
# baseline (speedup 1.0000x reference)
"""2-layer GCN encoder on 8 TRN2 NeuronCores (Bass/Tile).

Sharding: node (dst) sharding. Nodes are sorted by a 2D degree key
(max, heavy-side, min) over their per-table-half in-degrees and dealt
into 49 slots x 8 cores x 128 partitions so each slot's column budget
(maxA + maxB over the slot) stays close to the true degrees (~16% pad).

Layer math (exact up to fp reassociation; segment-sum commutes with the
dense projections, so BOTH layers gather RAW rows and project after
aggregation):
    agg1 = segsum(x[src]);  h = relu(agg1 @ W1_rel.T + x @ W1_root.T + b1)
    agg2 = segsum(h[src]);  out =    agg2 @ W2_rel.T + h @ W2_root.T + b2

Layer 1 gathers from a host-replicated slot-space table of RAW x rows
(ExternalInput) - no phase-1 compute and no AllGather gate: descriptor
generation and gather DMA start at t~10us. Layer 2 gathers raw h rows
(h is exactly 128 bf16 = 256B) from an AllGathered table.

Per slot: the gather list is degree-slotted so the message for node-slot
p always lands on SBUF partition p; identity matmuls accumulate the
message columns into a wide PSUM tile (NB lanes), a DVE fold reduces the
NB chunks, a PE transpose turns the aggregate into lhsT form, and one
matmul pair (root lhsT=xT/hT slot, rel lhsT=aggT) produces the layer
output directly in a [P, width] PSUM tile.

The critical resource is the SWDGE gather stream (~273-430 descs/us
across 4 queues); gathers round-robin the queues and idx tables load on
the scalar HWDGE ring so desc-gen starts as early as possible.
"""

import os
import sys

sys.path.insert(0, "/opt/trn_rl_repo")

import numpy as np

import concourse.bacc as bacc
import concourse.bass as bass
import concourse.mybir as mybir
import concourse.tile as tile
from concourse.bass_utils import run_bass_kernel_spmd
from concourse.masks import make_identity

P = 128
NCORES = 8
NB = 4  # edge-chunks accumulated per matmul (wide-PSUM lanes, NB*128 fp32 = 1 bank)

DEFAULT_CFG = dict(
    N=50000,   # real nodes
    F=96,      # input features
    H=128,     # hidden
    O=64,      # output features
    SLOTS=49,  # slots per core (NCORES*SLOTS*128 >= N)
    GMAX_COLS=24,   # gather-group width in columns (128 idxs each); groups
                    # are fixed-size chunks of the col space (may split slots)
    GBUFS=20,       # gather tiles in flight
)


def _derived(cfg):
    slots = cfg["SLOTS"]
    npc = slots * P              # node slots per core
    ntot = NCORES * npc          # total node slots
    half = ntot // 2             # table-half boundary (slot space)
    nhalf = cfg["N"] // 2        # real nodes per half (by original id)
    assert nhalf <= half - 1, "need at least one pad slot per half"
    assert half - 1 < 2**15, "table half must fit int16 indexing"
    return npc, ntot, half, nhalf


def _make_plan(src, dst, cfg):
    """Host-side planning. src/dst int32 arrays, self-loops removed."""
    N = cfg["N"]
    slots = cfg["SLOTS"]
    npc, ntot, half, nhalf = _derived(cfg)

    is_a = src < nhalf
    degA = np.bincount(dst[is_a], minlength=N).astype(np.int64)
    degB = np.bincount(dst[~is_a], minlength=N).astype(np.int64)

    node_dev = np.full(N, -1, np.int32)
    node_slot = np.full(N, -1, np.int32)
    node_part = np.full(N, -1, np.int32)
    node_of = np.full((NCORES, slots, P), -1, np.int64)
    KA = np.zeros(slots, np.int64)
    KB = np.zeros(slots, np.int64)
    pad_dsp = [None, None]  # (dev, slot, part) of the pad slot per half

    hcap = 4 * P  # nodes per (half, slot)
    for hf in (0, 1):
        nodes = np.arange(hf * nhalf, (hf + 1) * nhalf)
        # lexicographic (max, heavy-side, min): groups nodes whose (degA,
        # degB) profiles match in BOTH coordinates, so the per-slot
        # (maxA + maxB) column budget stays close to the per-node degree
        a, b = degA[nodes], degB[nodes]
        m, mn = np.maximum(a, b), np.minimum(a, b)
        key = (m * 2 + (a >= b)) * 64 + mn
        o = nodes[np.argsort(-key, kind="stable")]
        ranks = np.arange(len(o))
        s = ranks // hcap
        within = ranks % hcap
        d = hf * 4 + within // P
        p = within % P
        node_slot[o] = s
        node_dev[o] = d
        node_part[o] = p
        node_of[d, s, p] = o
        np.maximum.at(KA, s, degA[o])
        np.maximum.at(KB, s, degB[o])
        # first unused position in the half becomes the pad slot
        r0 = len(o)
        s0, w0 = r0 // hcap, r0 % hcap
        assert s0 < slots
        d0, p0 = hf * 4 + w0 // P, w0 % P
        pad_dsp[hf] = (int(d0), int(s0), int(p0))

    # relabel slots by consumption rank (lightest total degree first):
    # slot index == consumption order, so hr_loc rows are stored in
    # consumption order and the hr AllGather can be split into chunks
    # that fire as soon as their rows are written
    order = sorted(range(slots), key=lambda s: (int(KA[s] + KB[s]), s))
    rank = np.empty(slots, np.int64)
    rank[order] = np.arange(slots)
    node_slot = rank[node_slot].astype(np.int32)
    node_of = node_of[:, order, :]
    KA = KA[order]
    KB = KB[order]
    pad_pos = [
        d0 * npc + int(rank[s0]) * P + p0 for d0, s0, p0 in pad_dsp
    ]

    pos = node_dev.astype(np.int64) * npc + node_slot * P + node_part

    def layout_cols(K):
        colbase = np.concatenate([[0], np.cumsum(K)])
        return colbase, int(colbase[-1])

    colbaseA, totA = layout_cols(KA)
    colbaseB, totB = layout_cols(KB)
    LA = totA * P
    LB = totB * P

    def edge_fill(sel, colbase, Ltot, pad_val, sub):
        flat = np.full((NCORES, max(Ltot, 16)), pad_val, np.int64)
        pd = pos[dst[sel]]
        pv = pos[src[sel]] - sub
        eorder = np.argsort(pd, kind="stable")
        pd = pd[eorder]
        pv = pv[eorder]
        starts = np.searchsorted(pd, pd, side="left")
        rank = np.arange(len(pd)) - starts
        dev = pd // npc
        slot = (pd % npc) // P
        part = pd % P
        fpos = (colbase[slot] + rank) * P + part
        flat[dev, fpos] = pv
        assert flat.min() >= 0 and flat.max() < half
        # wrap: element i -> [i % 16, i // 16], then replicate block to 128 rows
        wrapped = flat.reshape(NCORES, -1, 16).transpose(0, 2, 1)
        return np.tile(wrapped, (1, 8, 1)).astype(np.int16)

    idxA = edge_fill(is_a, colbaseA, LA, pad_pos[0], 0)
    idxB = edge_fill(~is_a, colbaseB, LB, pad_pos[1] - half, half)

    def make_groups(K, colbase, total):
        # fixed-size chunks of the col space; slots may straddle chunks
        gmax = cfg["GMAX_COLS"]
        groups = [(c, min(c + gmax, total)) for c in range(0, total, gmax)]
        s2seg = [None] * slots
        for s in range(slots):
            segs = []
            c0 = int(colbase[s])
            rem = int(K[s])
            rel = 0
            while rem > 0:
                gid = c0 // gmax
                g0, g1 = groups[gid]
                take = min(rem, g1 - c0)
                segs.append((gid, c0 - g0, rel, take))
                c0 += take
                rel += take
                rem -= take
            s2seg[s] = segs
        return groups, s2seg

    groupsA, s2gA = make_groups(KA, colbaseA, totA)
    groupsB, s2gB = make_groups(KB, colbaseB, totB)

    meta = dict(
        cfg=dict(cfg),
        KA=[int(v) for v in KA],
        KB=[int(v) for v in KB],
        LA=max(LA, 16),
        LB=max(LB, 16),
        order=list(range(slots)),
        groupsA=groupsA,
        groupsB=groupsB,
        s2gA=s2gA,
        s2gB=s2gB,
    )
    return dict(
        meta=meta,
        node_dev=node_dev,
        node_slot=node_slot,
        node_part=node_part,
        node_of=node_of,
        pos=pos,
        idxA=idxA,
        idxB=idxB,
    )


def _bf16(a):
    import jax.numpy as jnp

    return np.asarray(jnp.asarray(a, dtype=jnp.bfloat16))


def _make_in_maps(plan, cfg, x, W1_rel, b1, W1_root, W2_rel, b2, W2_root):
    F, H, O = cfg["F"], cfg["H"], cfg["O"]
    slots = cfg["SLOTS"]
    npc, ntot, _, _ = _derived(cfg)
    node_of = plan["node_of"]

    # slot-space raw-x table, 128 bf16 lanes (cols F.. zero), replicated
    xfull = np.zeros((ntot, P), np.float32)
    xfull[plan["pos"][: x.shape[0]], :F] = x
    xfull_bf = _bf16(xfull)

    w1relT = np.zeros((F + 1, H), np.float32)
    w1relT[:F] = W1_rel.T
    w1rootT = np.zeros((F + 1, H), np.float32)
    w1rootT[:F] = W1_root.T
    w1rootT[F] = b1
    w2relT = np.ascontiguousarray(W2_rel.T, dtype=np.float32)  # [H, O]
    w2rootT = np.ascontiguousarray(W2_root.T, dtype=np.float32)  # [H, O]
    b2bc = np.ascontiguousarray(np.broadcast_to(b2, (P, O)), dtype=np.float32)

    in_maps = []
    for d in range(NCORES):
        members = node_of[d].reshape(-1)  # [npc]
        real = members >= 0
        xT = np.zeros((F + 1, npc), np.float32)
        xT[:F, real] = x[members[real]].T
        xT[F, real] = 1.0
        valid = np.zeros((P, slots), np.float32)
        valid[:, :] = real.reshape(slots, P).T
        in_maps.append(
            dict(
                xfull=xfull_bf,
                xT=_bf16(xT),
                w1relT=_bf16(w1relT),
                w1rootT=_bf16(w1rootT),
                w2relT=_bf16(w2relT),
                w2rootT=_bf16(w2rootT),
                b2bc=b2bc,
                valid=valid,
                idxA=np.ascontiguousarray(plan["idxA"][d]),
                idxB=np.ascontiguousarray(plan["idxB"][d]),
            )
        )
    return in_maps


def _build_nc(meta):
    cfg = meta["cfg"]
    F, H, O = cfg["F"], cfg["H"], cfg["O"]
    slots = cfg["SLOTS"]
    npc, ntot, half, _ = _derived(cfg)
    KA, KB = meta["KA"], meta["KB"]
    f32 = mybir.dt.float32
    bf16 = mybir.dt.bfloat16
    i16 = mybir.dt.int16
    RG = [list(range(NCORES))]

    nc = bacc.Bacc(
        "TRN2",
        target_bir_lowering=False,
        debug=False,
        num_devices=NCORES,
        num_swdge_queues=4,
    )
    xf_d = nc.dram_tensor("xfull", [ntot, P], bf16, kind="ExternalInput")
    xT_d = nc.dram_tensor("xT", [F + 1, npc], bf16, kind="ExternalInput")
    w1r_d = nc.dram_tensor("w1relT", [F + 1, H], bf16, kind="ExternalInput")
    w1o_d = nc.dram_tensor("w1rootT", [F + 1, H], bf16, kind="ExternalInput")
    w2r_d = nc.dram_tensor("w2relT", [H, O], bf16, kind="ExternalInput")
    w2o_d = nc.dram_tensor("w2rootT", [H, O], bf16, kind="ExternalInput")
    b2_d = nc.dram_tensor("b2bc", [P, O], f32, kind="ExternalInput")
    vld_d = nc.dram_tensor("valid", [P, slots], f32, kind="ExternalInput")
    ixA_d = nc.dram_tensor("idxA", [P, meta["LA"] // 16], i16, kind="ExternalInput")
    ixB_d = nc.dram_tensor("idxB", [P, meta["LB"] // 16], i16, kind="ExternalInput")
    out_d = nc.dram_tensor("out", [npc, O], f32, kind="ExternalOutput")

    hr_loc = nc.dram_tensor("hr_loc", [npc, H], bf16)
    hr_full = nc.dram_tensor("hr_full", [ntot, H], bf16, addr_space="Shared")

    GBUFS = cfg["GBUFS"]

    with tile.TileContext(nc) as tc:
        with (
            tc.tile_pool(name="const", bufs=1) as cp,
            tc.tile_pool(name="work", bufs=3) as wp,
            tc.tile_pool(name="gath", bufs=GBUFS) as gp,
            tc.tile_pool(name="psum", bufs=2, space="PSUM") as pp,
        ):
            # idx tables on the scalar HWDGE ring so they don't serialize
            # behind the other const loads (gathers need them first)
            ixA = cp.tile([P, meta["LA"] // 16], i16, tag="ixA")
            nc.scalar.dma_start(out=ixA[:], in_=ixA_d[:])
            ixB = cp.tile([P, meta["LB"] // 16], i16, tag="ixB")
            nc.scalar.dma_start(out=ixB[:], in_=ixB_d[:])

            def load_const(tag, dram, shape, dtype):
                t = cp.tile(shape, dtype, tag=tag)
                nc.sync.dma_start(out=t[:], in_=dram[:])
                return t

            w1r = load_const("w1r", w1r_d, [F + 1, H], bf16)
            w1o = load_const("w1o", w1o_d, [F + 1, H], bf16)
            w2r = load_const("w2r", w2r_d, [H, O], bf16)
            w2o = load_const("w2o", w2o_d, [H, O], bf16)
            b2 = load_const("b2", b2_d, [P, O], f32)
            vld = load_const("vld", vld_d, [P, slots], f32)
            xt = load_const("xt", xT_d, [F + 1, npc], bf16)

            ident = cp.tile([P, P], bf16, tag="ident")
            make_identity(nc, ident[:])
            ident32 = cp.tile([P, P], f32, tag="ident32")
            make_identity(nc, ident32[:])
            hT = cp.tile([P, npc], bf16, tag="hT")

            slot_order = meta["order"]

            class GatherLayer:
                def __init__(self, table):
                    self.table = table
                    # emission list ordered by first consuming slot
                    self.glist = []
                    seen = set()
                    for s in slot_order:
                        for st in (0, 1):
                            for gid, _off, _rel, _take in (
                                meta["s2gA"] if st == 0 else meta["s2gB"]
                            )[s]:
                                if (st, gid) not in seen:
                                    seen.add((st, gid))
                                    self.glist.append((st, gid))
                    self.gindex = {g: i for i, g in enumerate(self.glist)}
                    self.tiles = {}
                    self.next_emit = 0

                def emit_prep(self):
                    i = self.next_emit
                    st, gid = self.glist[i]
                    groups = meta["groupsA"] if st == 0 else meta["groupsB"]
                    c0, c1 = groups[gid]
                    L = (c1 - c0) * P
                    ix = ixA if st == 0 else ixB
                    half_ap = (
                        self.table[:half, :] if st == 0 else self.table[half:, :]
                    )
                    t = gp.tile([P, cfg["GMAX_COLS"] * H], bf16, tag="g")
                    nc.gpsimd.dma_gather(
                        out_ap=t[:, : (c1 - c0) * H].rearrange(
                            "p (c e) -> p c e", e=H
                        ),
                        in_ap=half_ap,
                        idxs_ap=ix[:, c0 * 8 : c1 * 8],
                        num_idxs=L,
                        num_idxs_reg=L,
                        elem_size=H,
                        single_packet=(L <= 1024),
                        queue_num=i % 4,
                    )
                    self.tiles[(st, gid)] = t
                    self.next_emit += 1

                def ensure(self, st, gid):
                    i = self.gindex[(st, gid)]
                    while self.next_emit <= i:
                        self.emit_prep()
                    return self.tiles[(st, gid)]

            # ---- software-pipelined layer loop -----------------------
            # Per slot the work chains PE -> DVE -> PE -> ACT -> PE across
            # in-order engines; emitted naively every slot pays the full
            # cross-engine latency (head-of-line on each engine). Emitting
            # with a stage skew (A(i) | B/C(i-1) | D(i-2)) keeps every
            # engine's queue supplied with already-runnable work.

            def stage_a(gl, s):
                """Identity-matmul accumulate of the slot's gathered
                message columns into a wide PSUM tile. The widest batch
                goes first: its start=True initializes every chunk any
                other batch accumulates into."""
                ps = pp.tile([P, NB * H], f32, tag="ps_big")
                batches = []
                for st in (0, 1):
                    for gid, goff, _rel, take in (
                        meta["s2gA"] if st == 0 else meta["s2gB"]
                    )[s]:
                        t = gl.ensure(st, gid)
                        for c0 in range(0, take, NB):
                            nb = min(NB, take - c0)
                            batches.append((t, goff + c0, nb))
                maxnb = max(b[2] for b in batches)
                wi = next(j for j, b in enumerate(batches) if b[2] == maxnb)
                batches[0], batches[wi] = batches[wi], batches[0]
                for i, (t, c0, nb) in enumerate(batches):
                    nc.tensor.matmul(
                        ps[:, : nb * H],
                        lhsT=ident[:],
                        rhs=t[:, c0 * H : (c0 + nb) * H],
                        start=(i == 0),
                        stop=(i == len(batches) - 1),
                    )
                return ps, maxnb

            def stage_bc(state):
                """DVE-fold the NB chunks, PE-transpose, cast to bf16."""
                ps, maxnb = state
                z = wp.tile([P, H], f32, tag="fold")
                nc.vector.tensor_copy(z[:], ps[:, :H])
                for j in range(1, maxnb):
                    nc.vector.tensor_tensor(
                        out=z[:], in0=z[:], in1=ps[:, j * H : (j + 1) * H],
                        op=mybir.AluOpType.add,
                    )
                pt = pp.tile([P, P], f32, tag="ps_tr")
                nc.tensor.transpose(pt[:], z[:], ident32[:])
                at = wp.tile([P, P], bf16, tag="aggT")
                nc.scalar.activation(
                    at[:], pt[:], mybir.ActivationFunctionType.Copy
                )
                return at

            def stage_d1(s, at):
                zp = pp.tile([P, H], f32, tag="ps_z")
                nc.tensor.matmul(
                    zp[:], lhsT=xt[:, s * P : (s + 1) * P], rhs=w1o[:],
                    start=True, stop=False,
                )
                nc.tensor.matmul(
                    zp[:], lhsT=at[: F + 1, :], rhs=w1r[:],
                    start=False, stop=True,
                )
                h = wp.tile([P, H], bf16, tag="hstage")
                # relu(z)*v == relu(z*v) for v in {0,1}: fold the pad-node
                # mask into the activation's per-partition scale
                nc.scalar.activation(
                    h[:], zp[:], mybir.ActivationFunctionType.Relu,
                    scale=vld[:, s : s + 1],
                )
                # h rows ARE the layer-2 gather table (raw-h gather)
                nc.sync.dma_start(out=hr_loc[s * P : (s + 1) * P, :], in_=h[:])
                pt2 = pp.tile([P, P], bf16, tag="ps_trb")
                nc.tensor.transpose(pt2[:], h[:], ident[:])
                nc.vector.tensor_copy(hT[:, s * P : (s + 1) * P], pt2[:])

            def stage_d2(s, at):
                zp = pp.tile([P, H], f32, tag="ps_z")
                nc.tensor.matmul(
                    zp[:, :O], lhsT=hT[:, s * P : (s + 1) * P], rhs=w2o[:],
                    start=True, stop=False,
                )
                nc.tensor.matmul(
                    zp[:, :O], lhsT=at[:], rhs=w2r[:], start=False, stop=True,
                )
                ot = wp.tile([P, O], f32, tag="small")
                nc.vector.tensor_tensor(
                    out=ot[:], in0=zp[:, :O], in1=b2[:], op=mybir.AluOpType.add
                )
                nc.sync.dma_start(out=out_d[s * P : (s + 1) * P, :], in_=ot[:])

            def layer_loop(gl, stage_d):
                acc = {}
                agg = {}
                n = len(slot_order)
                for i in range(n + 2):
                    if i < n:
                        acc[i] = stage_a(gl, slot_order[i])
                    if 1 <= i <= n:
                        agg[i - 1] = stage_bc(acc.pop(i - 1))
                    if i >= 2:
                        stage_d(slot_order[i - 2], agg.pop(i - 2))

            # ---- layer 1: gathers raw x rows; no table dependency
            gl1 = GatherLayer(xf_d)
            gl2 = GatherLayer(hr_full)
            layer_loop(gl1, stage_d1)

            # ---- hr table AllGather, then layer 2 (gathers raw h rows)
            nc.gpsimd.collective_compute(
                "AllGather",
                mybir.AluOpType.bypass,
                replica_groups=RG,
                ins=[hr_loc[:]],
                outs=[hr_full[:]],
            )
            layer_loop(gl2, stage_d2)

    nc.compile()
    return nc


_NC_CACHE = {}


def _meta_key(meta):
    return repr(
        (
            meta["cfg"],
            meta["KA"],
            meta["KB"],
            meta["groupsA"],
            meta["groupsB"],
        )
    )


def _run(inputs, cfg=None, trace=False):
    cfg = dict(DEFAULT_CFG if cfg is None else cfg)
    x = np.ascontiguousarray(np.asarray(inputs["x"], np.float32))
    ei = np.asarray(inputs["edge_index"])
    src = ei[0].astype(np.int64)
    dst = ei[1].astype(np.int64)
    keep = src != dst
    src = src[keep].astype(np.int32)
    dst = dst[keep].astype(np.int32)

    plan = _make_plan(src, dst, cfg)
    key = _meta_key(plan["meta"])
    if key not in _NC_CACHE:
        _NC_CACHE[key] = _build_nc(plan["meta"])
    nc = _NC_CACHE[key]

    in_maps = _make_in_maps(
        plan,
        cfg,
        x,
        np.asarray(inputs["W1_rel"], np.float32),
        np.asarray(inputs["b1"], np.float32),
        np.asarray(inputs["W1_root"], np.float32),
        np.asarray(inputs["W2_rel"], np.float32),
        np.asarray(inputs["b2"], np.float32),
        np.asarray(inputs["W2_root"], np.float32),
    )
    res = run_bass_kernel_spmd(
        nc, in_maps, list(range(NCORES)), trace=trace
    )

    N, O = cfg["N"], cfg["O"]
    out = np.empty((N, O), np.float32)
    local = plan["node_slot"] * P + plan["node_part"]
    for d in range(NCORES):
        sel = plan["node_dev"] == d
        out[sel] = res.results[d]["out"][local[sel]]
    return out, res


def kernel(**inputs) -> np.ndarray:
    out, _ = _run(inputs)
    return out


# revision 25
# speedup vs baseline: 1.1847x; 1.1847x over previous
"""2-layer GCN encoder on 8 TRN2 NeuronCores (Bass/Tile).

Sharding: node (dst) sharding. Nodes are sorted by a 2D degree key
(max, heavy-side, min) over their per-table-half in-degrees and dealt
into 49 slots x 8 cores x 128 partitions so each slot's column budget
(maxA + maxB over the slot) stays close to the true degrees (~16% pad).

Layer math (exact up to fp reassociation; segment-sum commutes with the
dense projections, so BOTH layers gather RAW rows and project after
aggregation):
    agg1 = segsum(x[src]);  h = relu(agg1 @ W1_rel.T + x @ W1_root.T + b1)
    agg2 = segsum(h[src]);  out =    agg2 @ W2_rel.T + h @ W2_root.T + b2

Layer 1 gathers from a host-replicated slot-space table of RAW x rows
(ExternalInput) - no phase-1 compute and no AllGather gate: descriptor
generation and gather DMA start at t~10us. Layer 2 gathers raw h rows
(h is exactly 128 bf16 = 256B) from an AllGathered table.

Per slot: the gather list is degree-slotted so the message for node-slot
p always lands on SBUF partition p; identity matmuls accumulate the
message columns into a wide PSUM tile (NB lanes), a DVE fold reduces the
NB chunks, a PE transpose turns the aggregate into lhsT form, and one
matmul pair (root lhsT=xT/hT slot, rel lhsT=aggT) produces the layer
output directly in a [P, width] PSUM tile.

The critical resource is the SWDGE gather stream (~273-430 descs/us
across 4 queues); gathers round-robin the queues and idx tables load on
the scalar HWDGE ring so desc-gen starts as early as possible.
"""

import os
import sys

sys.path.insert(0, "/opt/trn_rl_repo")

import numpy as np

import concourse.bacc as bacc
import concourse.bass as bass
import concourse.mybir as mybir
import concourse.tile as tile
from concourse.bass_utils import run_bass_kernel_spmd
from concourse.masks import make_identity

P = 128
NCORES = 8
NB = 4  # edge-chunks accumulated per matmul (wide-PSUM lanes, NB*128 fp32 = 1 bank)

DEFAULT_CFG = dict(
    N=50000,   # real nodes
    F=96,      # input features
    H=128,     # hidden
    O=64,      # output features
    SLOTS=49,  # slots per core (NCORES*SLOTS*128 >= N)
    GMAX_COLS=16,   # gather-group width in columns (128 idxs each); groups
                    # are fixed-size chunks of the col space (may split slots)
    GBUFS=24,       # gather tiles in flight
)


def _derived(cfg):
    slots = cfg["SLOTS"]
    npc = slots * P              # node slots per core
    ntot = NCORES * npc          # total node slots
    half = ntot // 2             # table-half boundary (slot space)
    nhalf = cfg["N"] // 2        # real nodes per half (by original id)
    assert nhalf <= half - 1, "need at least one pad slot per half"
    assert half - 1 < 2**15, "table half must fit int16 indexing"
    return npc, ntot, half, nhalf


def _make_plan(src, dst, cfg):
    """Host-side planning. src/dst int32 arrays, self-loops removed."""
    N = cfg["N"]
    slots = cfg["SLOTS"]
    npc, ntot, half, nhalf = _derived(cfg)

    is_a = src < nhalf
    degA = np.bincount(dst[is_a], minlength=N).astype(np.int64)
    degB = np.bincount(dst[~is_a], minlength=N).astype(np.int64)

    node_dev = np.full(N, -1, np.int32)
    node_slot = np.full(N, -1, np.int32)
    node_part = np.full(N, -1, np.int32)
    node_of = np.full((NCORES, slots, P), -1, np.int64)
    KA = np.zeros(slots, np.int64)
    KB = np.zeros(slots, np.int64)
    pad_dsp = [None, None]  # (dev, slot, part) of the pad slot per half

    hcap = 4 * P  # nodes per (half, slot)
    for hf in (0, 1):
        nodes = np.arange(hf * nhalf, (hf + 1) * nhalf)
        # lexicographic (max, heavy-side, min): groups nodes whose (degA,
        # degB) profiles match in BOTH coordinates, so the per-slot
        # (maxA + maxB) column budget stays close to the per-node degree
        a, b = degA[nodes], degB[nodes]
        m, mn = np.maximum(a, b), np.minimum(a, b)
        key = (m * 2 + (a >= b)) * 64 + mn
        o = nodes[np.argsort(-key, kind="stable")]
        ranks = np.arange(len(o))
        s = ranks // hcap
        within = ranks % hcap
        d = hf * 4 + within // P
        p = within % P
        node_slot[o] = s
        node_dev[o] = d
        node_part[o] = p
        node_of[d, s, p] = o
        np.maximum.at(KA, s, degA[o])
        np.maximum.at(KB, s, degB[o])
        # first unused position in the half becomes the pad slot
        r0 = len(o)
        s0, w0 = r0 // hcap, r0 % hcap
        assert s0 < slots
        d0, p0 = hf * 4 + w0 // P, w0 % P
        pad_dsp[hf] = (int(d0), int(s0), int(p0))

    # relabel slots by consumption rank (lightest total degree first):
    # slot index == consumption order, so hr_loc rows are stored in
    # consumption order and the hr AllGather can be split into chunks
    # that fire as soon as their rows are written
    order = sorted(range(slots), key=lambda s: (int(KA[s] + KB[s]), s))
    rank = np.empty(slots, np.int64)
    rank[order] = np.arange(slots)
    node_slot = rank[node_slot].astype(np.int32)
    node_of = node_of[:, order, :]
    KA = KA[order]
    KB = KB[order]
    pad_pos = [
        d0 * npc + int(rank[s0]) * P + p0 for d0, s0, p0 in pad_dsp
    ]

    pos = node_dev.astype(np.int64) * npc + node_slot * P + node_part

    def layout_cols(K):
        colbase = np.concatenate([[0], np.cumsum(K)])
        return colbase, int(colbase[-1])

    colbaseA, totA = layout_cols(KA)
    colbaseB, totB = layout_cols(KB)
    LA = totA * P
    LB = totB * P

    def edge_fill(sel, colbase, Ltot, pad_val, sub):
        flat = np.full((NCORES, max(Ltot, 16)), pad_val, np.int64)
        pd = pos[dst[sel]]
        pv = pos[src[sel]] - sub
        eorder = np.argsort(pd, kind="stable")
        pd = pd[eorder]
        pv = pv[eorder]
        starts = np.searchsorted(pd, pd, side="left")
        rank = np.arange(len(pd)) - starts
        dev = pd // npc
        slot = (pd % npc) // P
        part = pd % P
        fpos = (colbase[slot] + rank) * P + part
        flat[dev, fpos] = pv
        assert flat.min() >= 0 and flat.max() < half
        # wrap: element i -> [i % 16, i // 16], then replicate block to 128 rows
        wrapped = flat.reshape(NCORES, -1, 16).transpose(0, 2, 1)
        return np.tile(wrapped, (1, 8, 1)).astype(np.int16)

    idxA = edge_fill(is_a, colbaseA, LA, pad_pos[0], 0)
    idxB = edge_fill(~is_a, colbaseB, LB, pad_pos[1] - half, half)

    def make_groups(K, colbase, total):
        # fixed-size chunks of the col space; slots may straddle chunks
        gmax = cfg["GMAX_COLS"]
        groups = [(c, min(c + gmax, total)) for c in range(0, total, gmax)]
        s2seg = [None] * slots
        for s in range(slots):
            segs = []
            c0 = int(colbase[s])
            rem = int(K[s])
            rel = 0
            while rem > 0:
                gid = c0 // gmax
                g0, g1 = groups[gid]
                take = min(rem, g1 - c0)
                segs.append((gid, c0 - g0, rel, take))
                c0 += take
                rel += take
                rem -= take
            s2seg[s] = segs
        return groups, s2seg

    groupsA, s2gA = make_groups(KA, colbaseA, totA)
    groupsB, s2gB = make_groups(KB, colbaseB, totB)

    meta = dict(
        cfg=dict(cfg),
        KA=[int(v) for v in KA],
        KB=[int(v) for v in KB],
        LA=max(LA, 16),
        LB=max(LB, 16),
        order=list(range(slots)),
        groupsA=groupsA,
        groupsB=groupsB,
        s2gA=s2gA,
        s2gB=s2gB,
    )
    return dict(
        meta=meta,
        node_dev=node_dev,
        node_slot=node_slot,
        node_part=node_part,
        node_of=node_of,
        pos=pos,
        idxA=idxA,
        idxB=idxB,
    )


def _bf16(a):
    import jax.numpy as jnp

    return np.asarray(jnp.asarray(a, dtype=jnp.bfloat16))


def _make_in_maps(plan, cfg, x, W1_rel, b1, W1_root, W2_rel, b2, W2_root):
    F, H, O = cfg["F"], cfg["H"], cfg["O"]
    slots = cfg["SLOTS"]
    npc, ntot, _, _ = _derived(cfg)
    node_of = plan["node_of"]

    # slot-space raw-x table, 128 bf16 lanes (cols F.. zero), replicated
    xfull = np.zeros((ntot, P), np.float32)
    xfull[plan["pos"][: x.shape[0]], :F] = x
    xfull_bf = _bf16(xfull)

    w1relT = np.zeros((F + 1, H), np.float32)
    w1relT[:F] = W1_rel.T
    w1rootT = np.zeros((F + 1, H), np.float32)
    w1rootT[:F] = W1_root.T
    w1rootT[F] = b1
    w2relT = np.ascontiguousarray(W2_rel.T, dtype=np.float32)  # [H, O]
    w2rootT = np.ascontiguousarray(W2_root.T, dtype=np.float32)  # [H, O]
    b2bc = np.ascontiguousarray(np.broadcast_to(b2, (P, O)), dtype=np.float32)

    in_maps = []
    for d in range(NCORES):
        members = node_of[d].reshape(-1)  # [npc]
        real = members >= 0
        xT = np.zeros((F + 1, npc), np.float32)
        xT[:F, real] = x[members[real]].T
        xT[F, real] = 1.0
        valid = np.zeros((P, slots), np.float32)
        valid[:, :] = real.reshape(slots, P).T
        in_maps.append(
            dict(
                xfull=xfull_bf,
                xT=_bf16(xT),
                w1relT=_bf16(w1relT),
                w1rootT=_bf16(w1rootT),
                w2relT=_bf16(w2relT),
                w2rootT=_bf16(w2rootT),
                b2bc=b2bc,
                valid=valid,
                idxA=np.ascontiguousarray(plan["idxA"][d]),
                idxB=np.ascontiguousarray(plan["idxB"][d]),
            )
        )
    return in_maps


def _build_nc(meta):
    cfg = meta["cfg"]
    F, H, O = cfg["F"], cfg["H"], cfg["O"]
    slots = cfg["SLOTS"]
    npc, ntot, half, _ = _derived(cfg)
    KA, KB = meta["KA"], meta["KB"]
    f32 = mybir.dt.float32
    bf16 = mybir.dt.bfloat16
    i16 = mybir.dt.int16
    RG = [list(range(NCORES))]

    nc = bacc.Bacc(
        "TRN2",
        target_bir_lowering=False,
        debug=False,
        num_devices=NCORES,
        num_swdge_queues=4,
    )
    xf_d = nc.dram_tensor("xfull", [ntot, P], bf16, kind="ExternalInput")
    xT_d = nc.dram_tensor("xT", [F + 1, npc], bf16, kind="ExternalInput")
    w1r_d = nc.dram_tensor("w1relT", [F + 1, H], bf16, kind="ExternalInput")
    w1o_d = nc.dram_tensor("w1rootT", [F + 1, H], bf16, kind="ExternalInput")
    w2r_d = nc.dram_tensor("w2relT", [H, O], bf16, kind="ExternalInput")
    w2o_d = nc.dram_tensor("w2rootT", [H, O], bf16, kind="ExternalInput")
    b2_d = nc.dram_tensor("b2bc", [P, O], f32, kind="ExternalInput")
    vld_d = nc.dram_tensor("valid", [P, slots], f32, kind="ExternalInput")
    ixA_d = nc.dram_tensor("idxA", [P, meta["LA"] // 16], i16, kind="ExternalInput")
    ixB_d = nc.dram_tensor("idxB", [P, meta["LB"] // 16], i16, kind="ExternalInput")
    out_d = nc.dram_tensor("out", [npc, O], f32, kind="ExternalOutput")

    hr_loc = nc.dram_tensor("hr_loc", [npc, H], bf16)
    hr_full = nc.dram_tensor("hr_full", [ntot, H], bf16, addr_space="Shared")

    GBUFS = cfg["GBUFS"]

    with tile.TileContext(nc) as tc:
        with (
            tc.tile_pool(name="const", bufs=1) as cp,
            tc.tile_pool(name="work", bufs=3) as wp,
            tc.tile_pool(name="gath", bufs=GBUFS) as gp,
            tc.tile_pool(name="psum", bufs=2, space="PSUM") as pp,
        ):
            # idx tables on the scalar HWDGE ring so they don't serialize
            # behind the other const loads (gathers need them first)
            ixA = cp.tile([P, meta["LA"] // 16], i16, tag="ixA")
            nc.scalar.dma_start(out=ixA[:], in_=ixA_d[:])
            ixB = cp.tile([P, meta["LB"] // 16], i16, tag="ixB")
            nc.scalar.dma_start(out=ixB[:], in_=ixB_d[:])

            def load_const(tag, dram, shape, dtype):
                t = cp.tile(shape, dtype, tag=tag)
                nc.sync.dma_start(out=t[:], in_=dram[:])
                return t

            w1r = load_const("w1r", w1r_d, [F + 1, H], bf16)
            w1o = load_const("w1o", w1o_d, [F + 1, H], bf16)
            w2r = load_const("w2r", w2r_d, [H, O], bf16)
            w2o = load_const("w2o", w2o_d, [H, O], bf16)
            b2 = load_const("b2", b2_d, [P, O], f32)
            vld = load_const("vld", vld_d, [P, slots], f32)
            xt = load_const("xt", xT_d, [F + 1, npc], bf16)

            ident = cp.tile([P, P], bf16, tag="ident")
            make_identity(nc, ident[:])
            ident32 = cp.tile([P, P], f32, tag="ident32")
            make_identity(nc, ident32[:])
            hT = cp.tile([P, npc], bf16, tag="hT")

            slot_order = meta["order"]

            class GatherLayer:
                def __init__(self, table):
                    self.table = table
                    # emission list ordered by first consuming slot
                    self.glist = []
                    seen = set()
                    for s in slot_order:
                        for st in (0, 1):
                            for gid, _off, _rel, _take in (
                                meta["s2gA"] if st == 0 else meta["s2gB"]
                            )[s]:
                                if (st, gid) not in seen:
                                    seen.add((st, gid))
                                    self.glist.append((st, gid))
                    self.gindex = {g: i for i, g in enumerate(self.glist)}
                    self.tiles = {}
                    self.next_emit = 0

                def emit_prep(self):
                    i = self.next_emit
                    st, gid = self.glist[i]
                    groups = meta["groupsA"] if st == 0 else meta["groupsB"]
                    c0, c1 = groups[gid]
                    L = (c1 - c0) * P
                    ix = ixA if st == 0 else ixB
                    half_ap = (
                        self.table[:half, :] if st == 0 else self.table[half:, :]
                    )
                    t = gp.tile([P, cfg["GMAX_COLS"] * H], bf16, tag="g")
                    nc.gpsimd.dma_gather(
                        out_ap=t[:, : (c1 - c0) * H].rearrange(
                            "p (c e) -> p c e", e=H
                        ),
                        in_ap=half_ap,
                        idxs_ap=ix[:, c0 * 8 : c1 * 8],
                        num_idxs=L,
                        num_idxs_reg=L,
                        elem_size=H,
                        single_packet=(L <= 1024),
                        queue_num=i % 4,
                    )
                    self.tiles[(st, gid)] = t
                    self.next_emit += 1

                def ensure(self, st, gid):
                    i = self.gindex[(st, gid)]
                    while self.next_emit <= i:
                        self.emit_prep()
                    return self.tiles[(st, gid)]

            # ---- software-pipelined layer loop -----------------------
            # Per slot the work chains PE -> DVE -> PE -> ACT -> PE across
            # in-order engines; emitted naively every slot pays the full
            # cross-engine latency (head-of-line on each engine). Emitting
            # with a stage skew (A(i) | B/C(i-1) | D(i-2)) keeps every
            # engine's queue supplied with already-runnable work.

            def stage_a(gl, s):
                """Identity-matmul accumulate of the slot's gathered
                message columns into a wide PSUM tile. The widest batch
                goes first: its start=True initializes every chunk any
                other batch accumulates into."""
                ps = pp.tile([P, NB * H], f32, tag="ps_big")
                batches = []
                for st in (0, 1):
                    for gid, goff, _rel, take in (
                        meta["s2gA"] if st == 0 else meta["s2gB"]
                    )[s]:
                        t = gl.ensure(st, gid)
                        for c0 in range(0, take, NB):
                            nb = min(NB, take - c0)
                            batches.append((t, goff + c0, nb))
                maxnb = max(b[2] for b in batches)
                wi = next(j for j, b in enumerate(batches) if b[2] == maxnb)
                batches[0], batches[wi] = batches[wi], batches[0]
                for i, (t, c0, nb) in enumerate(batches):
                    nc.tensor.matmul(
                        ps[:, : nb * H],
                        lhsT=ident[:],
                        rhs=t[:, c0 * H : (c0 + nb) * H],
                        start=(i == 0),
                        stop=(i == len(batches) - 1),
                    )
                return ps, maxnb

            def stage_bc(state):
                """DVE-fold the NB chunks, PE-transpose, cast to bf16."""
                ps, maxnb = state
                z = wp.tile([P, H], f32, tag="fold")
                nc.vector.tensor_copy(z[:], ps[:, :H])
                for j in range(1, maxnb):
                    nc.vector.tensor_tensor(
                        out=z[:], in0=z[:], in1=ps[:, j * H : (j + 1) * H],
                        op=mybir.AluOpType.add,
                    )
                pt = pp.tile([P, P], f32, tag="ps_tr")
                nc.tensor.transpose(pt[:], z[:], ident32[:])
                at = wp.tile([P, P], bf16, tag="aggT")
                nc.scalar.activation(
                    at[:], pt[:], mybir.ActivationFunctionType.Copy
                )
                return at

            def stage_d1(s, at):
                zp = pp.tile([P, H], f32, tag="ps_z")
                nc.tensor.matmul(
                    zp[:], lhsT=xt[:, s * P : (s + 1) * P], rhs=w1o[:],
                    start=True, stop=False,
                )
                nc.tensor.matmul(
                    zp[:], lhsT=at[: F + 1, :], rhs=w1r[:],
                    start=False, stop=True,
                )
                h = wp.tile([P, H], bf16, tag="hstage")
                # relu(z)*v == relu(z*v) for v in {0,1}: fold the pad-node
                # mask into the activation's per-partition scale
                nc.scalar.activation(
                    h[:], zp[:], mybir.ActivationFunctionType.Relu,
                    scale=vld[:, s : s + 1],
                )
                # h rows ARE the layer-2 gather table (raw-h gather)
                nc.sync.dma_start(out=hr_loc[s * P : (s + 1) * P, :], in_=h[:])
                pt2 = pp.tile([P, P], bf16, tag="ps_trb")
                nc.tensor.transpose(pt2[:], h[:], ident[:])
                nc.vector.tensor_copy(hT[:, s * P : (s + 1) * P], pt2[:])

            def stage_d2(s, at):
                zp = pp.tile([P, H], f32, tag="ps_z")
                nc.tensor.matmul(
                    zp[:, :O], lhsT=hT[:, s * P : (s + 1) * P], rhs=w2o[:],
                    start=True, stop=False,
                )
                nc.tensor.matmul(
                    zp[:, :O], lhsT=at[:], rhs=w2r[:], start=False, stop=True,
                )
                ot = wp.tile([P, O], f32, tag="small")
                nc.vector.tensor_tensor(
                    out=ot[:], in0=zp[:, :O], in1=b2[:], op=mybir.AluOpType.add
                )
                nc.sync.dma_start(out=out_d[s * P : (s + 1) * P, :], in_=ot[:])

            def layer_loop(gl, stage_d):
                acc = {}
                agg = {}
                n = len(slot_order)
                for i in range(n + 2):
                    if i < n:
                        acc[i] = stage_a(gl, slot_order[i])
                    if 1 <= i <= n:
                        agg[i - 1] = stage_bc(acc.pop(i - 1))
                    if i >= 2:
                        stage_d(slot_order[i - 2], agg.pop(i - 2))

            # ---- layer 1: gathers raw x rows; no table dependency
            gl1 = GatherLayer(xf_d)
            gl2 = GatherLayer(hr_full)
            layer_loop(gl1, stage_d1)

            # ---- hr table AllGather, then layer 2 (gathers raw h rows)
            nc.gpsimd.collective_compute(
                "AllGather",
                mybir.AluOpType.bypass,
                replica_groups=RG,
                ins=[hr_loc[:]],
                outs=[hr_full[:]],
            )
            layer_loop(gl2, stage_d2)

    nc.compile()
    return nc


_NC_CACHE = {}


def _meta_key(meta):
    return repr(
        (
            meta["cfg"],
            meta["KA"],
            meta["KB"],
            meta["groupsA"],
            meta["groupsB"],
        )
    )


def _run(inputs, cfg=None, trace=False):
    cfg = dict(DEFAULT_CFG if cfg is None else cfg)
    x = np.ascontiguousarray(np.asarray(inputs["x"], np.float32))
    ei = np.asarray(inputs["edge_index"])
    src = ei[0].astype(np.int64)
    dst = ei[1].astype(np.int64)
    keep = src != dst
    src = src[keep].astype(np.int32)
    dst = dst[keep].astype(np.int32)

    plan = _make_plan(src, dst, cfg)
    key = _meta_key(plan["meta"])
    if key not in _NC_CACHE:
        _NC_CACHE[key] = _build_nc(plan["meta"])
    nc = _NC_CACHE[key]

    in_maps = _make_in_maps(
        plan,
        cfg,
        x,
        np.asarray(inputs["W1_rel"], np.float32),
        np.asarray(inputs["b1"], np.float32),
        np.asarray(inputs["W1_root"], np.float32),
        np.asarray(inputs["W2_rel"], np.float32),
        np.asarray(inputs["b2"], np.float32),
        np.asarray(inputs["W2_root"], np.float32),
    )
    res = run_bass_kernel_spmd(
        nc, in_maps, list(range(NCORES)), trace=trace
    )

    N, O = cfg["N"], cfg["O"]
    out = np.empty((N, O), np.float32)
    local = plan["node_slot"] * P + plan["node_part"]
    for d in range(NCORES):
        sel = plan["node_dev"] == d
        out[sel] = res.results[d]["out"][local[sel]]
    return out, res


def kernel(**inputs) -> np.ndarray:
    out, _ = _run(inputs)
    return out


# revision 26
# speedup vs baseline: 1.1931x; 1.0071x over previous
"""2-layer GCN encoder on 8 TRN2 NeuronCores (Bass/Tile).

Sharding: node (dst) sharding. Nodes are sorted by a 2D degree key
(max, heavy-side, min) over their per-table-half in-degrees and dealt
into 49 slots x 8 cores x 128 partitions so each slot's column budget
(maxA + maxB over the slot) stays close to the true degrees (~16% pad).

Layer math (exact up to fp reassociation; segment-sum commutes with the
dense projections, so BOTH layers gather RAW rows and project after
aggregation):
    agg1 = segsum(x[src]);  h = relu(agg1 @ W1_rel.T + x @ W1_root.T + b1)
    agg2 = segsum(h[src]);  out =    agg2 @ W2_rel.T + h @ W2_root.T + b2

Layer 1 gathers from a host-replicated slot-space table of RAW x rows
(ExternalInput) - no phase-1 compute and no AllGather gate: descriptor
generation and gather DMA start at t~10us. Layer 2 gathers raw h rows
(h is exactly 128 bf16 = 256B) from an AllGathered table.

Per slot: the gather list is degree-slotted so the message for node-slot
p always lands on SBUF partition p; identity matmuls accumulate the
message columns into a wide PSUM tile (NB lanes), a DVE fold reduces the
NB chunks, a PE transpose turns the aggregate into lhsT form, and one
matmul pair (root lhsT=xT/hT slot, rel lhsT=aggT) produces the layer
output directly in a [P, width] PSUM tile. The slot loop is emitted
software-pipelined (accumulate(i) | fold/transpose(i-1) | project(i-2))
so the in-order engines never head-of-line block on the current slot's
cross-engine chain; slots are relabeled so slot index == consumption
order (lightest first).

The critical resource is the SWDGE gather stream: Pool-engine
descriptor generation sustains ~430 descs/us standalone (~2.3 ns/desc;
one desc per edge, 256B elems) and ~290-340 descs/us in-kernel.
Gathers round-robin the 4 queues and idx tables load on the scalar
HWDGE ring so desc-gen starts as early as possible. Measured: 892 us
(baseline 1254 us); remaining known slack is ~140 us of Pool stalls in
the layer-2 stream plus the ~46 us hr AllGather (split-AG needs
chunk-contiguous collective outputs, which BIR rejects for interleaved
APs, and re-keying the int16 table halves to chunks is circular with
the degree-based slotting).
"""

import os
import sys

sys.path.insert(0, "/opt/trn_rl_repo")

import numpy as np

import concourse.bacc as bacc
import concourse.bass as bass
import concourse.mybir as mybir
import concourse.tile as tile
from concourse.bass_utils import run_bass_kernel_spmd
from concourse.masks import make_identity

P = 128
NCORES = 8
NB = 4  # edge-chunks accumulated per matmul (wide-PSUM lanes, NB*128 fp32 = 1 bank)

DEFAULT_CFG = dict(
    N=50000,   # real nodes
    F=96,      # input features
    H=128,     # hidden
    O=64,      # output features
    SLOTS=49,  # slots per core (NCORES*SLOTS*128 >= N)
    GMAX_COLS=16,   # gather-group width in columns (128 idxs each); groups
                    # are fixed-size chunks of the col space (may split slots)
    GBUFS=24,       # gather tiles in flight
)


def _derived(cfg):
    slots = cfg["SLOTS"]
    npc = slots * P              # node slots per core
    ntot = NCORES * npc          # total node slots
    half = ntot // 2             # table-half boundary (slot space)
    nhalf = cfg["N"] // 2        # real nodes per half (by original id)
    assert nhalf <= half - 1, "need at least one pad slot per half"
    assert half - 1 < 2**15, "table half must fit int16 indexing"
    return npc, ntot, half, nhalf


def _make_plan(src, dst, cfg):
    """Host-side planning. src/dst int32 arrays, self-loops removed."""
    N = cfg["N"]
    slots = cfg["SLOTS"]
    npc, ntot, half, nhalf = _derived(cfg)

    is_a = src < nhalf
    degA = np.bincount(dst[is_a], minlength=N).astype(np.int64)
    degB = np.bincount(dst[~is_a], minlength=N).astype(np.int64)

    node_dev = np.full(N, -1, np.int32)
    node_slot = np.full(N, -1, np.int32)
    node_part = np.full(N, -1, np.int32)
    node_of = np.full((NCORES, slots, P), -1, np.int64)
    KA = np.zeros(slots, np.int64)
    KB = np.zeros(slots, np.int64)
    pad_dsp = [None, None]  # (dev, slot, part) of the pad slot per half

    hcap = 4 * P  # nodes per (half, slot)
    for hf in (0, 1):
        nodes = np.arange(hf * nhalf, (hf + 1) * nhalf)
        # lexicographic (max, heavy-side, min): groups nodes whose (degA,
        # degB) profiles match in BOTH coordinates, so the per-slot
        # (maxA + maxB) column budget stays close to the per-node degree
        a, b = degA[nodes], degB[nodes]
        m, mn = np.maximum(a, b), np.minimum(a, b)
        key = (m * 2 + (a >= b)) * 64 + mn
        o = nodes[np.argsort(-key, kind="stable")]
        ranks = np.arange(len(o))
        s = ranks // hcap
        within = ranks % hcap
        d = hf * 4 + within // P
        p = within % P
        node_slot[o] = s
        node_dev[o] = d
        node_part[o] = p
        node_of[d, s, p] = o
        np.maximum.at(KA, s, degA[o])
        np.maximum.at(KB, s, degB[o])
        # first unused position in the half becomes the pad slot
        r0 = len(o)
        s0, w0 = r0 // hcap, r0 % hcap
        assert s0 < slots
        d0, p0 = hf * 4 + w0 // P, w0 % P
        pad_dsp[hf] = (int(d0), int(s0), int(p0))

    # relabel slots by consumption rank (lightest total degree first):
    # slot index == consumption order, so hr_loc rows are stored in
    # consumption order and the hr AllGather can be split into chunks
    # that fire as soon as their rows are written
    order = sorted(range(slots), key=lambda s: (int(KA[s] + KB[s]), s))
    rank = np.empty(slots, np.int64)
    rank[order] = np.arange(slots)
    node_slot = rank[node_slot].astype(np.int32)
    node_of = node_of[:, order, :]
    KA = KA[order]
    KB = KB[order]
    pad_pos = [
        d0 * npc + int(rank[s0]) * P + p0 for d0, s0, p0 in pad_dsp
    ]

    pos = node_dev.astype(np.int64) * npc + node_slot * P + node_part

    def layout_cols(K):
        colbase = np.concatenate([[0], np.cumsum(K)])
        return colbase, int(colbase[-1])

    colbaseA, totA = layout_cols(KA)
    colbaseB, totB = layout_cols(KB)
    LA = totA * P
    LB = totB * P

    def edge_fill(sel, colbase, Ltot, pad_val, sub):
        flat = np.full((NCORES, max(Ltot, 16)), pad_val, np.int64)
        pd = pos[dst[sel]]
        pv = pos[src[sel]] - sub
        eorder = np.argsort(pd, kind="stable")
        pd = pd[eorder]
        pv = pv[eorder]
        starts = np.searchsorted(pd, pd, side="left")
        rank = np.arange(len(pd)) - starts
        dev = pd // npc
        slot = (pd % npc) // P
        part = pd % P
        fpos = (colbase[slot] + rank) * P + part
        flat[dev, fpos] = pv
        assert flat.min() >= 0 and flat.max() < half
        # wrap: element i -> [i % 16, i // 16], then replicate block to 128 rows
        wrapped = flat.reshape(NCORES, -1, 16).transpose(0, 2, 1)
        return np.tile(wrapped, (1, 8, 1)).astype(np.int16)

    idxA = edge_fill(is_a, colbaseA, LA, pad_pos[0], 0)
    idxB = edge_fill(~is_a, colbaseB, LB, pad_pos[1] - half, half)

    def make_groups(K, colbase, total):
        # fixed-size chunks of the col space; slots may straddle chunks
        gmax = cfg["GMAX_COLS"]
        groups = [(c, min(c + gmax, total)) for c in range(0, total, gmax)]
        s2seg = [None] * slots
        for s in range(slots):
            segs = []
            c0 = int(colbase[s])
            rem = int(K[s])
            rel = 0
            while rem > 0:
                gid = c0 // gmax
                g0, g1 = groups[gid]
                take = min(rem, g1 - c0)
                segs.append((gid, c0 - g0, rel, take))
                c0 += take
                rel += take
                rem -= take
            s2seg[s] = segs
        return groups, s2seg

    groupsA, s2gA = make_groups(KA, colbaseA, totA)
    groupsB, s2gB = make_groups(KB, colbaseB, totB)

    meta = dict(
        cfg=dict(cfg),
        KA=[int(v) for v in KA],
        KB=[int(v) for v in KB],
        LA=max(LA, 16),
        LB=max(LB, 16),
        order=list(range(slots)),
        groupsA=groupsA,
        groupsB=groupsB,
        s2gA=s2gA,
        s2gB=s2gB,
    )
    return dict(
        meta=meta,
        node_dev=node_dev,
        node_slot=node_slot,
        node_part=node_part,
        node_of=node_of,
        pos=pos,
        idxA=idxA,
        idxB=idxB,
    )


def _bf16(a):
    import jax.numpy as jnp

    return np.asarray(jnp.asarray(a, dtype=jnp.bfloat16))


def _make_in_maps(plan, cfg, x, W1_rel, b1, W1_root, W2_rel, b2, W2_root):
    F, H, O = cfg["F"], cfg["H"], cfg["O"]
    slots = cfg["SLOTS"]
    npc, ntot, _, _ = _derived(cfg)
    node_of = plan["node_of"]

    # slot-space raw-x table, 128 bf16 lanes (cols F.. zero), replicated
    xfull = np.zeros((ntot, P), np.float32)
    xfull[plan["pos"][: x.shape[0]], :F] = x
    xfull_bf = _bf16(xfull)

    w1relT = np.zeros((F + 1, H), np.float32)
    w1relT[:F] = W1_rel.T
    w1rootT = np.zeros((F + 1, H), np.float32)
    w1rootT[:F] = W1_root.T
    w1rootT[F] = b1
    w2relT = np.ascontiguousarray(W2_rel.T, dtype=np.float32)  # [H, O]
    w2rootT = np.ascontiguousarray(W2_root.T, dtype=np.float32)  # [H, O]
    b2bc = np.ascontiguousarray(np.broadcast_to(b2, (P, O)), dtype=np.float32)

    in_maps = []
    for d in range(NCORES):
        members = node_of[d].reshape(-1)  # [npc]
        real = members >= 0
        xT = np.zeros((F + 1, npc), np.float32)
        xT[:F, real] = x[members[real]].T
        xT[F, real] = 1.0
        valid = np.zeros((P, slots), np.float32)
        valid[:, :] = real.reshape(slots, P).T
        in_maps.append(
            dict(
                xfull=xfull_bf,
                xT=_bf16(xT),
                w1relT=_bf16(w1relT),
                w1rootT=_bf16(w1rootT),
                w2relT=_bf16(w2relT),
                w2rootT=_bf16(w2rootT),
                b2bc=b2bc,
                valid=valid,
                idxA=np.ascontiguousarray(plan["idxA"][d]),
                idxB=np.ascontiguousarray(plan["idxB"][d]),
            )
        )
    return in_maps


def _build_nc(meta):
    cfg = meta["cfg"]
    F, H, O = cfg["F"], cfg["H"], cfg["O"]
    slots = cfg["SLOTS"]
    npc, ntot, half, _ = _derived(cfg)
    KA, KB = meta["KA"], meta["KB"]
    f32 = mybir.dt.float32
    bf16 = mybir.dt.bfloat16
    i16 = mybir.dt.int16
    RG = [list(range(NCORES))]

    nc = bacc.Bacc(
        "TRN2",
        target_bir_lowering=False,
        debug=False,
        num_devices=NCORES,
        num_swdge_queues=4,
    )
    xf_d = nc.dram_tensor("xfull", [ntot, P], bf16, kind="ExternalInput")
    xT_d = nc.dram_tensor("xT", [F + 1, npc], bf16, kind="ExternalInput")
    w1r_d = nc.dram_tensor("w1relT", [F + 1, H], bf16, kind="ExternalInput")
    w1o_d = nc.dram_tensor("w1rootT", [F + 1, H], bf16, kind="ExternalInput")
    w2r_d = nc.dram_tensor("w2relT", [H, O], bf16, kind="ExternalInput")
    w2o_d = nc.dram_tensor("w2rootT", [H, O], bf16, kind="ExternalInput")
    b2_d = nc.dram_tensor("b2bc", [P, O], f32, kind="ExternalInput")
    vld_d = nc.dram_tensor("valid", [P, slots], f32, kind="ExternalInput")
    ixA_d = nc.dram_tensor("idxA", [P, meta["LA"] // 16], i16, kind="ExternalInput")
    ixB_d = nc.dram_tensor("idxB", [P, meta["LB"] // 16], i16, kind="ExternalInput")
    out_d = nc.dram_tensor("out", [npc, O], f32, kind="ExternalOutput")

    hr_loc = nc.dram_tensor("hr_loc", [npc, H], bf16)
    hr_full = nc.dram_tensor("hr_full", [ntot, H], bf16, addr_space="Shared")

    GBUFS = cfg["GBUFS"]

    with tile.TileContext(nc) as tc:
        with (
            tc.tile_pool(name="const", bufs=1) as cp,
            tc.tile_pool(name="work", bufs=3) as wp,
            tc.tile_pool(name="gath", bufs=GBUFS) as gp,
            tc.tile_pool(name="psum", bufs=2, space="PSUM") as pp,
        ):
            # idx tables on the scalar HWDGE ring so they don't serialize
            # behind the other const loads (gathers need them first)
            ixA = cp.tile([P, meta["LA"] // 16], i16, tag="ixA")
            nc.scalar.dma_start(out=ixA[:], in_=ixA_d[:])
            ixB = cp.tile([P, meta["LB"] // 16], i16, tag="ixB")
            nc.scalar.dma_start(out=ixB[:], in_=ixB_d[:])

            def load_const(tag, dram, shape, dtype):
                t = cp.tile(shape, dtype, tag=tag)
                nc.sync.dma_start(out=t[:], in_=dram[:])
                return t

            w1r = load_const("w1r", w1r_d, [F + 1, H], bf16)
            w1o = load_const("w1o", w1o_d, [F + 1, H], bf16)
            w2r = load_const("w2r", w2r_d, [H, O], bf16)
            w2o = load_const("w2o", w2o_d, [H, O], bf16)
            b2 = load_const("b2", b2_d, [P, O], f32)
            vld = load_const("vld", vld_d, [P, slots], f32)
            xt = load_const("xt", xT_d, [F + 1, npc], bf16)

            ident = cp.tile([P, P], bf16, tag="ident")
            make_identity(nc, ident[:])
            ident32 = cp.tile([P, P], f32, tag="ident32")
            make_identity(nc, ident32[:])
            hT = cp.tile([P, npc], bf16, tag="hT")

            slot_order = meta["order"]

            class GatherLayer:
                def __init__(self, table):
                    self.table = table
                    # emission list ordered by first consuming slot
                    self.glist = []
                    seen = set()
                    for s in slot_order:
                        for st in (0, 1):
                            for gid, _off, _rel, _take in (
                                meta["s2gA"] if st == 0 else meta["s2gB"]
                            )[s]:
                                if (st, gid) not in seen:
                                    seen.add((st, gid))
                                    self.glist.append((st, gid))
                    self.gindex = {g: i for i, g in enumerate(self.glist)}
                    self.tiles = {}
                    self.next_emit = 0

                def emit_prep(self):
                    i = self.next_emit
                    st, gid = self.glist[i]
                    groups = meta["groupsA"] if st == 0 else meta["groupsB"]
                    c0, c1 = groups[gid]
                    L = (c1 - c0) * P
                    ix = ixA if st == 0 else ixB
                    half_ap = (
                        self.table[:half, :] if st == 0 else self.table[half:, :]
                    )
                    t = gp.tile([P, cfg["GMAX_COLS"] * H], bf16, tag="g")
                    nc.gpsimd.dma_gather(
                        out_ap=t[:, : (c1 - c0) * H].rearrange(
                            "p (c e) -> p c e", e=H
                        ),
                        in_ap=half_ap,
                        idxs_ap=ix[:, c0 * 8 : c1 * 8],
                        num_idxs=L,
                        num_idxs_reg=L,
                        elem_size=H,
                        single_packet=(L <= 1024),
                        queue_num=i % 4,
                    )
                    self.tiles[(st, gid)] = t
                    self.next_emit += 1

                def ensure(self, st, gid):
                    i = self.gindex[(st, gid)]
                    while self.next_emit <= i:
                        self.emit_prep()
                    return self.tiles[(st, gid)]

            # ---- software-pipelined layer loop -----------------------
            # Per slot the work chains PE -> DVE -> PE -> ACT -> PE across
            # in-order engines; emitted naively every slot pays the full
            # cross-engine latency (head-of-line on each engine). Emitting
            # with a stage skew (A(i) | B/C(i-1) | D(i-2)) keeps every
            # engine's queue supplied with already-runnable work.

            def stage_a(gl, s):
                """Identity-matmul accumulate of the slot's gathered
                message columns into a wide PSUM tile. The widest batch
                goes first: its start=True initializes every chunk any
                other batch accumulates into."""
                ps = pp.tile([P, NB * H], f32, tag="ps_big")
                batches = []
                for st in (0, 1):
                    for gid, goff, _rel, take in (
                        meta["s2gA"] if st == 0 else meta["s2gB"]
                    )[s]:
                        t = gl.ensure(st, gid)
                        for c0 in range(0, take, NB):
                            nb = min(NB, take - c0)
                            batches.append((t, goff + c0, nb))
                maxnb = max(b[2] for b in batches)
                wi = next(j for j, b in enumerate(batches) if b[2] == maxnb)
                batches[0], batches[wi] = batches[wi], batches[0]
                for i, (t, c0, nb) in enumerate(batches):
                    nc.tensor.matmul(
                        ps[:, : nb * H],
                        lhsT=ident[:],
                        rhs=t[:, c0 * H : (c0 + nb) * H],
                        start=(i == 0),
                        stop=(i == len(batches) - 1),
                    )
                return ps, maxnb

            def stage_bc(state):
                """DVE-fold the NB chunks, PE-transpose, cast to bf16."""
                ps, maxnb = state
                z = wp.tile([P, H], f32, tag="fold")
                nc.vector.tensor_copy(z[:], ps[:, :H])
                for j in range(1, maxnb):
                    nc.vector.tensor_tensor(
                        out=z[:], in0=z[:], in1=ps[:, j * H : (j + 1) * H],
                        op=mybir.AluOpType.add,
                    )
                pt = pp.tile([P, P], f32, tag="ps_tr")
                nc.tensor.transpose(pt[:], z[:], ident32[:])
                at = wp.tile([P, P], bf16, tag="aggT")
                nc.scalar.activation(
                    at[:], pt[:], mybir.ActivationFunctionType.Copy
                )
                return at

            def stage_d1(s, at):
                zp = pp.tile([P, H], f32, tag="ps_z")
                nc.tensor.matmul(
                    zp[:], lhsT=xt[:, s * P : (s + 1) * P], rhs=w1o[:],
                    start=True, stop=False,
                )
                nc.tensor.matmul(
                    zp[:], lhsT=at[: F + 1, :], rhs=w1r[:],
                    start=False, stop=True,
                )
                h = wp.tile([P, H], bf16, tag="hstage")
                # relu(z)*v == relu(z*v) for v in {0,1}: fold the pad-node
                # mask into the activation's per-partition scale
                nc.scalar.activation(
                    h[:], zp[:], mybir.ActivationFunctionType.Relu,
                    scale=vld[:, s : s + 1],
                )
                # h rows ARE the layer-2 gather table (raw-h gather)
                nc.sync.dma_start(out=hr_loc[s * P : (s + 1) * P, :], in_=h[:])
                pt2 = pp.tile([P, P], bf16, tag="ps_trb")
                nc.tensor.transpose(pt2[:], h[:], ident[:])
                nc.vector.tensor_copy(hT[:, s * P : (s + 1) * P], pt2[:])

            def stage_d2(s, at):
                zp = pp.tile([P, H], f32, tag="ps_z")
                nc.tensor.matmul(
                    zp[:, :O], lhsT=hT[:, s * P : (s + 1) * P], rhs=w2o[:],
                    start=True, stop=False,
                )
                nc.tensor.matmul(
                    zp[:, :O], lhsT=at[:], rhs=w2r[:], start=False, stop=True,
                )
                ot = wp.tile([P, O], f32, tag="small")
                nc.vector.tensor_tensor(
                    out=ot[:], in0=zp[:, :O], in1=b2[:], op=mybir.AluOpType.add
                )
                nc.sync.dma_start(out=out_d[s * P : (s + 1) * P, :], in_=ot[:])

            def layer_loop(gl, stage_d):
                acc = {}
                agg = {}
                n = len(slot_order)
                for i in range(n + 2):
                    if i < n:
                        acc[i] = stage_a(gl, slot_order[i])
                    if 1 <= i <= n:
                        agg[i - 1] = stage_bc(acc.pop(i - 1))
                    if i >= 2:
                        stage_d(slot_order[i - 2], agg.pop(i - 2))

            # ---- layer 1: gathers raw x rows; no table dependency
            gl1 = GatherLayer(xf_d)
            gl2 = GatherLayer(hr_full)
            layer_loop(gl1, stage_d1)

            # ---- hr table AllGather, then layer 2 (gathers raw h rows)
            nc.gpsimd.collective_compute(
                "AllGather",
                mybir.AluOpType.bypass,
                replica_groups=RG,
                ins=[hr_loc[:]],
                outs=[hr_full[:]],
            )
            layer_loop(gl2, stage_d2)

    nc.compile()
    return nc


_NC_CACHE = {}


def _meta_key(meta):
    return repr(
        (
            meta["cfg"],
            meta["KA"],
            meta["KB"],
            meta["groupsA"],
            meta["groupsB"],
        )
    )


def _run(inputs, cfg=None, trace=False):
    cfg = dict(DEFAULT_CFG if cfg is None else cfg)
    x = np.ascontiguousarray(np.asarray(inputs["x"], np.float32))
    ei = np.asarray(inputs["edge_index"])
    src = ei[0].astype(np.int64)
    dst = ei[1].astype(np.int64)
    keep = src != dst
    src = src[keep].astype(np.int32)
    dst = dst[keep].astype(np.int32)

    plan = _make_plan(src, dst, cfg)
    key = _meta_key(plan["meta"])
    if key not in _NC_CACHE:
        _NC_CACHE[key] = _build_nc(plan["meta"])
    nc = _NC_CACHE[key]

    in_maps = _make_in_maps(
        plan,
        cfg,
        x,
        np.asarray(inputs["W1_rel"], np.float32),
        np.asarray(inputs["b1"], np.float32),
        np.asarray(inputs["W1_root"], np.float32),
        np.asarray(inputs["W2_rel"], np.float32),
        np.asarray(inputs["b2"], np.float32),
        np.asarray(inputs["W2_root"], np.float32),
    )
    res = run_bass_kernel_spmd(
        nc, in_maps, list(range(NCORES)), trace=trace
    )

    N, O = cfg["N"], cfg["O"]
    out = np.empty((N, O), np.float32)
    local = plan["node_slot"] * P + plan["node_part"]
    for d in range(NCORES):
        sel = plan["node_dev"] == d
        out[sel] = res.results[d]["out"][local[sel]]
    return out, res


def kernel(**inputs) -> np.ndarray:
    out, _ = _run(inputs)
    return out
